# revision 49
# baseline (speedup 1.0000x reference)
"""Trainium2 Bass kernel for Attention_concat (separable PAM attention).

Math (per batch b, N = H*W = 4096):
    eq = x^T wq_eff + bq_eff        (wq_eff = Wq^T wq, wq = Wc[:inter])
    ek = x^T wk_eff + bk_eff
    attention[i, j] = (eq[i] + ek[j]) / N          (rank-structured, never built)
    out = v @ attention, v = Wv x + bv
    y = gamma * out + x
      = x + A[c] + Bv[c] * ekn[m]                  (rank-1 over spatial m)
with
    u = x @ 1, t = x @ eqn (eqn = eq - bq_eff), E_nb = wq_eff . u
    s_nb = Wv (t + bq_eff u),  V_nb = Wv u
    g = gamma / N
    Bv = g V_nb + g N bv
    A  = g (s_nb + bk_eff V_nb) + bv (g E_nb + g N (bq_eff + bk_eff))

Precision: the attention term is ~1.5e-4 of |y| (x dominates), so the whole
reduction pipeline runs in bf16 on the PE at full rate; only the final
y = x + psum add uses exact fp32 x.

Sharding: 2 cores per batch; each core receives the full x[b] (column-permuted
so its own half of the spatial positions comes first), computes the global
reductions redundantly, and writes the first 2048 output columns.

Engine split per 1024-column superblock: HWDGE DMA streams x (fp32 for the
core's own half, host-cast bf16 for the other half); ACT fuses the bf16 cast
with the u accumulation (activation Copy + accum_out); PE broadcasts eq
directly into PSUM via a matmul whose stationary operand is wq_eff replicated
along the free dim; DVE fuses the t multiply+reduce in one
scalar_tensor_tensor (accum_out); ek rows land straight in the RC tile for
phase C, where one matmul per 512 block computes Bv⊗ek + A⊗ones and one DVE
add applies x. Dummy matmuls on a memset tile warm the PE HAM clock gate
during the initial DMA wait. Module-level workarounds: this container's
walrus accepts only one sync-wait per instruction, so extra waits are hoisted
onto single-wait NoOps at BIR level, and the Tile tail drain is rebuilt the
same way.
"""

import json as _json

import numpy as np

import concourse.bass as bass
import concourse.bass2jax as _b2j
import concourse.bass_utils as _bu
import concourse.mybir as mybir
import concourse.tile as tile
from concourse.bass_utils import run_bass_kernel_spmd
from concourse.tile_rust import add_dep_helper
from concourse.vector_clock import ScopedClock, VectorClock

B, C, H, W = 4, 256, 64, 64
N = H * W            # 4096
INTER = C // 4       # 64
NCORES = 8
HALF = N // 2        # 2048 output columns per core
SUP = 4              # 1024-column DMA superblocks
F32 = mybir.dt.float32
BF16 = mybir.dt.bfloat16
AX = mybir.AxisListType
OP = mybir.AluOpType
ACTF = mybir.ActivationFunctionType


def _split_multi_waits(bir: dict) -> dict:
    """The nix walrus accepts only ONE sync-wait command per instruction.
    Hoist extra waits onto preceding single-wait NoOps on the same engine
    (sequencers execute in program order, so semantics are unchanged)."""
    ctr = 0
    for fn in bir.get("functions", []):
        for blk in fn.get("blocks", []):
            insts = blk.get("instructions")
            if not insts:
                continue
            out = []
            for inst in insts:
                si = inst.get("sync_info") or {}
                waits = si.get("on_wait") or []
                if len(waits) > 1 and inst.get("engine", "Unassigned") != "Unassigned":
                    for w in waits[:-1]:
                        ctr += 1
                        out.append({
                            "debug": inst.get("debug", 0),
                            "engine": inst["engine"],
                            "ins": [], "outs": [],
                            "name": f"{inst['name']}-ws{ctr}",
                            "opcode": "NoOp",
                            "sync_info": {"on_update": [], "on_wait": [w]},
                        })
                    si["on_wait"] = [waits[-1]]
                out.append(inst)
            blk["instructions"] = out
    return bir


_WAIT_SPLIT_DONE = False


def install_wait_split():
    global _WAIT_SPLIT_DONE
    if _WAIT_SPLIT_DONE:
        return
    orig = _bu.compile_bir_kernel

    def wrapped(bir_json, *a, **kw):
        d = _json.loads(bir_json)
        _split_multi_waits(d)
        return orig(_json.dumps(d).encode(), *a, **kw)

    _bu.compile_bir_kernel = wrapped
    _b2j.compile_bir_kernel = wrapped
    _WAIT_SPLIT_DONE = True


class SplitDrainTileContext(tile.TileContext):
    """Tail fix for the same 1-wait walrus limit: park the global-clock waits
    on single-wait Nops spread across all five engines (they wait in
    parallel), then a wait-free drain + the usual barrier/reset."""

    def _drain_and_barrier(self, tick_clock, wait_clock):
        gc = tick_clock.global_clock
        nprocs = len(gc)
        engines = [self.nc.sync, self.nc.vector, self.nc.scalar,
                   self.nc.gpsimd, self.nc.tensor]
        idx = 0
        for proc in range(nprocs):
            if gc[proc] > 0:
                eng = engines[idx % len(engines)]
                idx += 1
                nop = eng.nop(nofuse=True, hint=f"tail_wait_p{proc}")
                vc = VectorClock([0] * nprocs)
                vc.require_at_least(proc, gc[proc])
                wait_clock.add_sem_waits(nop.ins, ScopedClock({None: vc}))
        self.nc.sync.drain()
        self.nc.all_engine_barrier()
        assert self.sems is not None
        popped = self.nc._tile_sem_poison_stack.pop()
        assert popped is self._sem_poison
        self.nc.clear_and_free_semaphores(list(self.sems.allocated().values()))
        self.nc.all_engine_barrier()


def build_kernel(g: float, bq_eff: float, bk_eff: float):
    """Build the per-core Bass program. g = gamma/N."""
    nc = bass.Bass()
    xin = nc.dram_tensor("xin", [C, HALF], F32, kind="ExternalInput")
    xbh = nc.dram_tensor("xbh", [C, HALF], BF16, kind="ExternalInput")
    wqk = nc.dram_tensor("wqk", [128, 2, 2], BF16, kind="ExternalInput")
    wqrep = nc.dram_tensor("wqrep", [128, 2, 128], BF16, kind="ExternalInput")
    wvt = nc.dram_tensor("wvt", [128, 2, C], BF16, kind="ExternalInput")
    bvrow = nc.dram_tensor("bvrow", [1, C], F32, kind="ExternalInput")
    bvgn = nc.dram_tensor("bvgn", [1, C], F32, kind="ExternalInput")
    yout = nc.dram_tensor("yout", [C, HALF], F32, kind="ExternalOutput")

    with SplitDrainTileContext(nc) as tc:
        with (
            tc.tile_pool(name="persist", bufs=1) as pp,
            tc.tile_pool(name="trash", bufs=4) as tp,
            tc.tile_pool(name="ypool", bufs=4) as yp,
            tc.tile_pool(name="prows", bufs=2, space="PSUM") as prows,
            tc.tile_pool(name="pbig", bufs=3, space="PSUM") as pbig,
        ):
            # --- persistent tiles -------------------------------------------------
            # fp32 x only for the core's own half (exact final add)
            xt = [[pp.tile([128, 1024], F32, tag=f"x{q}_{k}", name=f"x{q}_{k}")
                   for k in range(2)] for q in range(2)]
            xbf = [[pp.tile([128, 1024], BF16, tag=f"xb{q}_{k}", name=f"xb{q}_{k}")
                    for k in range(SUP)] for q in range(2)]
            wqk_sb = pp.tile([128, 2, 2], BF16, tag="wqk")
            wqrep_sb = pp.tile([128, 2, 128], BF16, tag="wqrep")
            wvt_sb = pp.tile([128, 2, C], BF16, tag="wvt")
            bv_sb = pp.tile([1, C], F32, tag="bv")
            bvgn_sb = pp.tile([1, C], F32, tag="bvgn")
            RC = pp.tile([2, HALF], BF16, tag="RC")    # row0 = ek, row1 = ones
            ONES1 = pp.tile([1, HALF], BF16, tag="ONES1")
            AB = pp.tile([2, C], BF16, tag="AB")       # row0 = Bv, row1 = A
            tacc = pp.tile([128, 2, SUP], F32, tag="tacc")
            uacc = pp.tile([128, 2, SUP], F32, tag="uacc")
            t2 = pp.tile([128, 2], F32, tag="t2")
            u2 = pp.tile([128, 2], F32, tag="u2")
            tu = pp.tile([128, 2, 2], F32, tag="tu")
            tub = pp.tile([128, 2, 2], BF16, tag="tub")
            u2b = pp.tile([128, 2], BF16, tag="u2b")
            s_sb = pp.tile([1, C], F32, tag="s_sb")
            v_sb = pp.tile([1, C], F32, tag="v_sb")
            e_sb = pp.tile([1, 1], F32, tag="e_sb")
            sc_sb = pp.tile([1, 1], F32, tag="sc_sb")
            aa = pp.tile([1, C], BF16, tag="aa")
            abv = pp.tile([1, C], BF16, tag="abv")
            tm1 = pp.tile([1, C], F32, tag="tm1")
            tm2 = pp.tile([1, C], F32, tag="tm2")

            nc.gpsimd.memset(ONES1, 1.0)
            # RC row1 = ones (cross-partition row move via DMA)
            nc.sync.dma_start(out=RC[1:2, :], in_=ONES1[0:1, :])
            # first compute block's data goes out before the weights so it
            # finishes the SDMA round-robin soonest
            for q in range(2):
                nc.sync.dma_start(out=xt[q][0],
                                  in_=xin[128 * q:128 * (q + 1), 0:1024])
            nc.sync.dma_start(out=wqk_sb, in_=wqk[:, :, :])
            nc.sync.dma_start(out=wqrep_sb, in_=wqrep[:, :, :])
            nc.sync.dma_start(out=wvt_sb, in_=wvt[:, :, :])
            nc.sync.dma_start(out=bv_sb, in_=bvrow[:, :])
            nc.sync.dma_start(out=bvgn_sb, in_=bvgn[:, :])

            # PE warm-up: dummy matmuls on a memset tile (no DMA dependency)
            # during the DMA wait, so the HAM clock gate reaches 2.4 GHz
            # before the real matmuls arrive.
            wusrc = pp.tile([128, 512], BF16, tag="wusrc")
            nc.vector.memset(wusrc, 0.5)
            for i in range(26):
                wu = pbig.tile([128, 512], F32, tag="big", name=f"wu{i}")
                nc.tensor.matmul(wu, wusrc[:, 0:128], wusrc,
                                 start=True, stop=True)

            # --- phase A: stream x, cast, eqb direct, ek rows, t/u ---------------
            for k in range(SUP):
                for q in range(2):
                    if k < 2:
                        # own half: fp32 load + fused bf16 cast + u accumulate
                        if k > 0:
                            nc.sync.dma_start(
                                out=xt[q][k],
                                in_=xin[128 * q:128 * (q + 1),
                                        1024 * k:1024 * (k + 1)],
                            )
                        nc.scalar.activation(
                            out=xbf[q][k], in_=xt[q][k], func=ACTF.Copy,
                            accum_out=uacc[:, q, k:k + 1],
                        )
                    else:
                        # other half: bf16 straight from the host, u only
                        nc.sync.dma_start(
                            out=xbf[q][k],
                            in_=xbh[128 * q:128 * (q + 1),
                                    1024 * (k - 2):1024 * (k - 1)],
                        )
                        tru = tp.tile([128, 1024], BF16, tag="prod")
                        nc.scalar.activation(
                            out=tru, in_=xbf[q][k], func=ACTF.Copy,
                            accum_out=uacc[:, q, k:k + 1],
                        )
                # eqb direct: lhsT = wq_eff replicated along the free dim, so
                # every output row i gets eq[n] — the partition broadcast comes
                # out of the matmul itself, no eq-row round trip needed.
                eqb = pbig.tile([128, 1024], F32, tag="big")
                for sub in range(2):
                    blk = slice(512 * sub, 512 * (sub + 1))
                    for q in range(2):
                        nc.tensor.matmul(eqb[:, blk], wqrep_sb[:, q, :],
                                         xbf[q][k][:, blk],
                                         start=(q == 0), stop=(q == 1))
                # ek row, only for the core's own output half (k < 2)
                if k < 2:
                    for sub in range(2):
                        blk = slice(512 * sub, 512 * (sub + 1))
                        gcol = slice(1024 * k + 512 * sub,
                                     1024 * k + 512 * sub + 512)
                        ekp = prows.tile([1, 512], F32, tag="qk")
                        for q in range(2):
                            nc.tensor.matmul(ekp, wqk_sb[:, q, 1:2],
                                             xbf[q][k][:, blk],
                                             start=(q == 0), stop=(q == 1))
                        nc.scalar.copy(out=RC[0:1, gcol], in_=ekp)
                for q in range(2):
                    # t-partial: fused multiply+reduce in one DVE pass
                    src = xt[q][k] if k < 2 else xbf[q][k]
                    prod = tp.tile([128, 1024], BF16, tag="prod")
                    nc.vector.scalar_tensor_tensor(
                        out=prod, in0=src, scalar=0.0, in1=eqb,
                        op0=OP.add, op1=OP.mult,
                        accum_out=tacc[:, q, k:k + 1],
                    )

            # --- tail: fold reductions into A/Bv rows ----------------------------
            for q in range(2):
                nc.vector.tensor_reduce(out=t2[:, q:q + 1], in_=tacc[:, q, :],
                                        axis=AX.X, op=OP.add)
                nc.vector.tensor_reduce(out=u2[:, q:q + 1], in_=uacc[:, q, :],
                                        axis=AX.X, op=OP.add)
                # tu[:,q,0] = t + bq_eff*u ; tu[:,q,1] = u
                nc.vector.tensor_scalar(out=tu[:, q, 1:2], in0=u2[:, q:q + 1],
                                        scalar1=bq_eff, scalar2=None, op0=OP.mult)
                nc.vector.tensor_tensor(out=tu[:, q, 0:1], in0=tu[:, q, 1:2],
                                        in1=t2[:, q:q + 1], op=OP.add)
                nc.vector.tensor_copy(out=tu[:, q, 1:2], in_=u2[:, q:q + 1])
                nc.vector.tensor_copy(out=tub[:, q, :], in_=tu[:, q, :])
                nc.vector.tensor_copy(out=u2b[:, q:q + 1], in_=u2[:, q:q + 1])

            ep = prows.tile([1, 1], F32, tag="qk")
            sp = prows.tile([1, C], F32, tag="qk")
            vp = prows.tile([1, C], F32, tag="qk")
            for q in range(2):
                nc.tensor.matmul(ep, u2b[:, q:q + 1], wqk_sb[:, q, 0:1],
                                 start=(q == 0), stop=(q == 1))
                nc.tensor.matmul(sp, tub[:, q, 0:1], wvt_sb[:, q, :],
                                 start=(q == 0), stop=(q == 1))
                nc.tensor.matmul(vp, tub[:, q, 1:2], wvt_sb[:, q, :],
                                 start=(q == 0), stop=(q == 1))
            nc.scalar.copy(out=e_sb, in_=ep)
            # wvt is pre-scaled by g on the host, so sp/vp rows are already
            # s2 = g*s_nb and V2 = g*V_nb.
            nc.scalar.copy(out=s_sb, in_=sp)
            nc.scalar.copy(out=v_sb, in_=vp)

            # sc = g*E_nb + g*N*(bq_eff + bk_eff)
            nc.scalar.activation(out=sc_sb, in_=e_sb, func=ACTF.Copy,
                                 bias=g * N * (bq_eff + bk_eff), scale=g)
            # A = s2 + bk_eff*V2 + bv*sc ; Bv = V2 + g*N*bv (bvgN from host)
            nc.vector.tensor_scalar(out=tm1, in0=v_sb, scalar1=bk_eff,
                                    scalar2=None, op0=OP.mult)
            nc.vector.tensor_tensor(out=tm1, in0=tm1, in1=s_sb, op=OP.add)
            nc.vector.tensor_scalar(out=tm2, in0=bv_sb, scalar1=sc_sb,
                                    scalar2=None, op0=OP.mult)
            nc.vector.tensor_tensor(out=aa, in0=tm1, in1=tm2, op=OP.add)
            nc.vector.tensor_tensor(out=abv, in0=v_sb, in1=bvgn_sb, op=OP.add)
            # AB rows: row0 = Bv (pairs with RC row0 = ek), row1 = A (ones)
            nc.sync.dma_start(out=AB[0:1, :], in_=abv[0:1, :])
            nc.sync.dma_start(out=AB[1:2, :], in_=aa[0:1, :])

            # --- phase C: y = x + A + Bv*ek over own half (first 2048 cols) ------
            for q in range(2):
                for k in range(2):
                    ys = yp.tile([128, 1024], F32, tag="y")
                    yps = pbig.tile([128, 1024], F32, tag="big")
                    for sub in range(2):
                        blk = slice(512 * sub, 512 * (sub + 1))
                        gcol = slice(1024 * k + 512 * sub, 1024 * k + 512 * sub + 512)
                        nc.tensor.matmul(yps[:, blk],
                                         AB[:, 128 * q:128 * (q + 1)],
                                         RC[0:2, gcol], start=True, stop=True)
                    nc.vector.tensor_tensor(out=ys, in0=xt[q][k],
                                            in1=yps, op=OP.add)
                    nc.sync.dma_start(
                        out=yout[128 * q:128 * (q + 1), 1024 * k:1024 * (k + 1)],
                        in_=ys,
                    )
    return nc


def host_prep(x, Wq, bq, Wk, bk, Wc, Wv, bv, gamma):
    """Fold weights on host; build per-core input maps."""
    x = np.asarray(x, dtype=np.float32)
    Wq = np.asarray(Wq, np.float32); bq = np.asarray(bq, np.float32)
    Wk = np.asarray(Wk, np.float32); bk = np.asarray(bk, np.float32)
    Wc = np.asarray(Wc, np.float32)
    Wv = np.asarray(Wv, np.float32); bv = np.asarray(bv, np.float32)
    gamma = float(np.asarray(gamma).reshape(-1)[0])

    wqv, wkv = Wc[:INTER], Wc[INTER:]
    wq_eff = (wqv @ Wq).astype(np.float32)          # [C]
    wk_eff = (wkv @ Wk).astype(np.float32)
    bq_eff = float(wqv @ bq)
    bk_eff = float(wkv @ bk)
    g = gamma / float(N)

    import ml_dtypes
    bf = ml_dtypes.bfloat16
    # wqk[p, q, 0] = wq_eff chunk q; wqk[p, q, 1] = wk_eff chunk q
    wqk_np = np.stack(
        [np.stack([wq_eff[:128], wk_eff[:128]], axis=1),
         np.stack([wq_eff[128:], wk_eff[128:]], axis=1)], axis=1).astype(bf)
    # wq_eff replicated along the output free dim for the direct-eqb matmul
    wqrep_np = np.broadcast_to(
        np.stack([wq_eff[:128], wq_eff[128:]], axis=1)[:, :, None].astype(bf),
        (128, 2, 128)).copy()
    # g folded into Wv so the s/V matmuls directly give g*s_nb, g*V_nb
    wvt_np = (g * Wv.T).reshape(2, 128, C).transpose(1, 0, 2).astype(bf)
    bvrow = bv.reshape(1, C)
    bvgn = (g * N * bv).reshape(1, C).astype(np.float32)

    xr = x.reshape(B, C, N)
    xbf_all = xr.astype(bf)
    in_maps = []
    for core in range(NCORES):
        b, half = core // 2, core % 2
        own = slice(HALF * half, HALF * (half + 1))
        other = slice(HALF * (1 - half), HALF * (2 - half))
        in_maps.append({
            "xin": np.ascontiguousarray(xr[b][:, own]),
            "xbh": np.ascontiguousarray(xbf_all[b][:, other]),
            "wqk": np.ascontiguousarray(wqk_np),
            "wqrep": np.ascontiguousarray(wqrep_np),
            "wvt": np.ascontiguousarray(wvt_np),
            "bvrow": np.ascontiguousarray(bvrow),
            "bvgn": np.ascontiguousarray(bvgn),
        })
    return in_maps, (g, bq_eff, bk_eff)


def assemble(results):
    """Stitch per-core halves into the full output [B, C, H, W]."""
    y = np.empty((B, C, N), dtype=np.float32)
    for core in range(NCORES):
        b, half = core // 2, core % 2
        y[b, :, HALF * half:HALF * (half + 1)] = results[core]["yout"]
    return y.reshape(B, C, H, W)


def kernel(**inputs):
    install_wait_split()
    in_maps, (g, bq_eff, bk_eff) = host_prep(**inputs)
    nc = build_kernel(g, bq_eff, bk_eff)
    res = run_bass_kernel_spmd(nc, in_maps, core_ids=list(range(NCORES)))
    return assemble(res.results)
